# revision 6
# baseline (speedup 1.0000x reference)
"""Sparse transposed-conv block (gather + per-offset GEMM + sync-BN + ReLU) on 8 TRN2 NeuronCores.

Strategy (data-parallel over output voxels, per the sharding hint):
 - Each core owns a contiguous block of M/8 output voxels; the full feats
   table is replicated (bf16, banked for int16 gather indices) and read with
   the bulk `dma_gather(transpose=True)` custom op, which lands gathered
   rows CHANNELS-ON-PARTITIONS — no PE transposes needed before the GEMM.
 - Host-side index prep only: per-shard voxels are sorted by
   (src-bank, kernel-offset); subgroup sizes are padded to 128 and to the
   max across cores so the single SPMD program fits every core's data.
   Pad slots gather a sentinel zero row, contributing exact zeros to the
   BN statistics.
 - Phase 1: transpose-gather (bf16) -> bf16 matmuls with [W_k|0]/[0|W_k]
   weight pairs accumulating a subtile-parity-packed [128, 512] PSUM
   supertile -> ACT copies it into an SBUF-resident fp16 pre-BN buffer
   while reduce-accumulating per-channel sums; a second ACT pass
   accumulates sums of squares.
 - Mid: [64,2] AllReduce across the 8 cores (sync-BN), scale/bias compute.
 - Phase 2: ACT fused relu(scale*x+bias) -> bf16 -> PE transpose back to
   voxel-major -> plain contiguous HWDGE writes into HBM laid out
   [128 part, voxel-block, 64ch]; the host un-permutes (each output voxel
   is written exactly once - the scatter is a collision-free permutation).
"""

import math
import os
import numpy as np
import ml_dtypes

import concourse.bass as bass
import concourse.bacc as bacc
import concourse.tile as tile
import concourse.mybir as mybir
from concourse import bass_utils
from concourse.masks import make_identity

P = 128
N_CORES = 8
BN_EPS = 1e-5

N_IN, M_FULL, CIN, COUT, KVOL = 200000, 600000, 128, 64, 4

BANK = 32767                 # real rows per bank (int16 sentinel at 32767)
BROWS = BANK + 1             # rows per bank incl sentinel
SUBS_PER_SUPER = 8           # 128-voxel subtiles per 1024-voxel supertile
SUPER = SUBS_PER_SUPER * P
MAX_OP = 896                 # max voxels per gather op: the SWDGE ring holds
                             # 1024 descriptors per queue and transpose-mode
                             # gathers emit ~1.125 descriptors per row
N_QUEUES = 4
SCAT_SUPERS = 2              # supertiles per contiguous output write (2KB/part)

MM_DT = mybir.dt.bfloat16
ACC_DT = mybir.dt.float16    # SBUF-resident pre-BN buffer dtype (values ~N(0,0.5))


def _wrap16(lst):
    """int16 index list -> [128, n/16] tile data (16-partition wrap,
    replicated for the 8 SWDGE cores)."""
    n = lst.shape[0]
    assert n % 16 == 0
    w = lst.reshape(n // 16, 16).T.astype(np.int16)   # [16, n/16]
    return np.tile(w, (8, 1))                          # [128, n/16]


def build_schedule(in_idx, kidx, n_cores, m_shard, kvol, n_in):
    """Returns (per-core gidx16 [C,128,NT*8], per-core sched-pos [C,m_shard],
    plan dict, NT)."""
    s_banks = math.ceil(n_in / BANK)

    counts = np.zeros((n_cores, s_banks, kvol), np.int64)
    orders = []
    for c in range(n_cores):
        r = np.arange(m_shard)
        k_sh = kidx[c * m_shard:(c + 1) * m_shard]
        src = in_idx[c * m_shard:(c + 1) * m_shard]
        sb = src // BANK
        order = np.lexsort((r, k_sh, sb))  # stable by (sb, k)
        orders.append(order)
        np.add.at(counts[c], (sb[order], k_sh[order]), 1)

    g_max = counts.max(axis=0)                       # [s_banks, kvol]
    g_pad = (np.ceil(g_max / P) * P).astype(np.int64)
    total = int(g_pad.sum())
    # extend the last nonzero subgroup so the schedule is store-tile-aligned
    batch = SCAT_SUPERS * SUPER
    total_al = math.ceil(total / batch) * batch
    nz = np.argwhere(g_pad > 0)
    ls, lk = nz[-1]
    g_pad[ls, lk] += total_al - total
    total = total_al
    nt = total // P

    # subgroup offsets in schedule order
    sg_off = np.zeros_like(g_pad)
    off = 0
    sg_list = []   # (sb, k, off, padded_len)
    for s in range(s_banks):
        for k in range(kvol):
            if g_pad[s, k] == 0:
                continue
            sg_off[s, k] = off
            sg_list.append((s, k, off, int(g_pad[s, k])))
            off += int(g_pad[s, k])

    # per-subtile k map -> per-supertile matmul runs
    sub_k = np.empty(nt, np.int64)
    for (s, k, o, ln) in sg_list:
        sub_k[o // P:(o + ln) // P] = k
    runs = []
    for u in range(total // SUPER):
        r = []
        ks = sub_k[u * SUBS_PER_SUPER:(u + 1) * SUBS_PER_SUPER]
        i = 0
        while i < SUBS_PER_SUPER:
            j = i
            while j < SUBS_PER_SUPER and ks[j] == ks[i]:
                j += 1
            r.append((int(ks[i]), i, j))
            i = j
        runs.append(r)

    # gather ops: src-bank-pure chunks (<= MAX_OP, 128-aligned); subgroups
    # sharing a src bank merge into one op up to the chunk limit
    gops = []   # (src_bank, sched_pos, n)
    for s in range(s_banks):
        ext = [(o, ln) for (s2, k, o, ln) in sg_list if s2 == s]
        if not ext:
            continue
        o0 = ext[0][0]
        ln = sum(e[1] for e in ext)
        assert o0 + ln == ext[-1][0] + ext[-1][1]
        p0 = o0
        while p0 < o0 + ln:
            n = min(MAX_OP, o0 + ln - p0)
            gops.append((s, p0, n))
            p0 += n

    # per-core int16 gather index lists + schedule positions
    gidx16 = np.empty((n_cores, P, nt * 8), np.int16)
    pos_all = np.empty((n_cores, m_shard), np.int64)
    for c in range(n_cores):
        order = orders[c]
        k_sh = kidx[c * m_shard:(c + 1) * m_shard]
        src = in_idx[c * m_shard:(c + 1) * m_shard]
        glist = np.full(total, BANK, np.int64)   # pad -> sentinel zero row
        k_o = k_sh[order]
        sb_o = (src // BANK)[order]
        key = sb_o * kvol + k_o
        uniq, inv, cnt = np.unique(key, return_inverse=True, return_counts=True)
        within = np.arange(m_shard) - np.concatenate([[0], np.cumsum(cnt)])[inv]
        pos = sg_off[sb_o, k_o] + within
        glist[pos] = (src % BANK)[order]
        pos_all[c][order] = pos
        gidx16[c] = _wrap16(glist)

    plan = dict(s_banks=s_banks, runs=runs, gops=gops, total=total)
    return gidx16, pos_all, plan, nt


def build_program(n_in, m_shard, nt, plan, n_cores):
    f32 = mybir.dt.float32
    i16 = mybir.dt.int16
    n_super = nt // SUBS_PER_SUPER
    s_banks = plan["s_banks"]
    runs, gops = plan["runs"], plan["gops"]

    nc = bacc.Bacc("TRN2", target_bir_lowering=False, debug=False,
                   num_devices=n_cores, num_swdge_queues=N_QUEUES)

    feats_d = nc.dram_tensor("feats", [s_banks * BROWS, CIN], MM_DT,
                             kind="ExternalInput")
    w_d = nc.dram_tensor("wcat", [CIN, KVOL * 2 * P], MM_DT, kind="ExternalInput")
    gb_d = nc.dram_tensor("gb", [COUT, 2], f32, kind="ExternalInput")
    gidx_d = nc.dram_tensor("gidx", [P, nt * 8], i16, kind="ExternalInput")
    out_d = nc.dram_tensor("out", [P, nt * COUT], MM_DT, kind="ExternalOutput")

    # static helper map: schedule subtile -> (gather op index, col offset in op)
    sub_op = {}
    for w, (s, p0, n) in enumerate(gops):
        for t in range(n // P):
            sub_op[p0 // P + t] = (w, t * P)

    with tile.TileContext(nc) as tc:
        with tc.tile_pool(name="const", bufs=1) as cpool, \
             tc.tile_pool(name="big", bufs=1) as big, \
             tc.tile_pool(name="gst", bufs=4) as gst_pool, \
             tc.tile_pool(name="gix", bufs=4) as gix_pool, \
             tc.tile_pool(name="sqn", bufs=2) as sqn_pool, \
             tc.tile_pool(name="store", bufs=2) as store_pool, \
             tc.tile_pool(name="small", bufs=1) as small, \
             tc.tile_pool(name="psA", bufs=3, space="PSUM") as psA, \
             tc.tile_pool(name="psB", bufs=2, space="PSUM") as psB, \
             tc.tile_pool(name="dram", bufs=2, space="DRAM") as dram:

            ident = cpool.tile([P, P], MM_DT)
            make_identity(nc, ident[:])
            w_sb = cpool.tile([CIN, KVOL * 2 * P], MM_DT)
            nc.sync.dma_start(out=w_sb[:], in_=w_d.ap())
            gb_sb = cpool.tile([COUT, 2], f32)
            nc.sync.dma_start(out=gb_sb[:], in_=gb_d.ap())

            out_all = big.tile([P, n_super * (SUPER // 2)], ACC_DT)
            macc = small.tile([P, n_super], f32)
            sacc = small.tile([P, n_super], f32)

            # ---------------- Phase 1 ----------------
            stage = {}   # op index -> staging tile [128 chan, n vox] bf16
            def issue_gather(w):
                s, p0, n = gops[w]
                gix = gix_pool.tile([P, MAX_OP // 16], i16, tag="gix")
                nc.sync.dma_start(out=gix[:, :n // 16],
                                  in_=gidx_d.ap()[:, p0 // 16:(p0 + n) // 16])
                gst = gst_pool.tile([P, MAX_OP], MM_DT, tag="gst")
                nc.gpsimd.dma_gather(
                    gst[:, :n].rearrange("p (o n) -> p o n", o=1),
                    feats_d.ap()[s * BROWS:(s + 1) * BROWS, :],
                    gix[:, :n // 16],
                    n, n, CIN,
                    transpose=True,
                    queue_num=w % N_QUEUES)
                stage[w] = gst

            next_op = 0
            for u in range(n_super):
                last_sub = (u + 1) * SUBS_PER_SUPER - 1
                while next_op < len(gops) and \
                        gops[next_op][1] // P <= last_sub:
                    issue_gather(next_op)
                    next_op += 1

                # out2[(c,j), blk*128 + p] = conv(voxel (2*blk+c)*128 + p)
                # start=True zeroes the whole 2KB PSUM bank (ZERO_REGION), so
                # only the first matmul of the supertile may set it; Tile
                # serializes same-bank ops in emission order.
                out2 = psA.tile([P, SUPER // 2], f32, tag="out2")
                mm_list = []   # (k, c, i0, nsub, op, col0)
                for (k, ss, se) in runs[u]:
                    for c in range(2):
                        subs = [t for t in range(ss, se) if t % 2 == c]
                        g = None
                        for t in subs:
                            w, col = sub_op[u * SUBS_PER_SUPER + t]
                            if g is not None and g[4] == w and \
                                    col == g[5] + g[3] * 2 * P:
                                g[3] += 1
                            else:
                                if g is not None:
                                    mm_list.append(tuple(g))
                                g = [k, c, t, 1, w, col]
                        if g is not None:
                            mm_list.append(tuple(g))
                for i, (k, c, t0, nsub, w, col0) in enumerate(mm_list):
                    gt = stage[w][:]
                    rhs = bass.AP(
                        gt.tensor, gt.offset + col0,
                        [gt.ap[0], [2 * P, nsub], [1, P]])
                    o0 = (t0 // 2) * P
                    nc.tensor.matmul(
                        out=out2[:, o0:o0 + nsub * P],
                        lhsT=w_sb[:, (k * 2 + c) * P:(k * 2 + c + 1) * P],
                        rhs=rhs,
                        start=(i == 0), stop=(i == len(mm_list) - 1),
                        skip_group_check=True)

                nc.scalar.activation(
                    out=out_all[:, u * 512:(u + 1) * 512], in_=out2[:],
                    func=mybir.ActivationFunctionType.Copy,
                    accum_out=macc[:, u:u + 1])
                sq_sb = sqn_pool.tile([P, SUPER // 2], f32, tag="sqn")
                nc.scalar.activation(
                    out=sq_sb[:], in_=out2[:],
                    func=mybir.ActivationFunctionType.Square,
                    accum_out=sacc[:, u:u + 1])

            # ---------------- stats + AllReduce ----------------
            kraw = os.environ.get("KRAW", "0") == "1"
            stats = small.tile([P, 2], f32)
            nc.vector.reduce_sum(out=stats[:, 0:1], in_=macc[:],
                                 axis=mybir.AxisListType.X)
            nc.vector.reduce_sum(out=stats[:, 1:2], in_=sacc[:],
                                 axis=mybir.AxisListType.X)
            fold = small.tile([COUT, 2], f32)
            nc.sync.dma_start(out=fold[:], in_=stats[COUT:2 * COUT, :])
            sums = small.tile([COUT, 2], f32)
            nc.vector.tensor_add(out=sums[:], in0=stats[0:COUT, :], in1=fold[:])

            knocoll = os.environ.get("KNOCOLL", "0") == "1"
            if knocoll:
                red = sums
                inv_m = 1.0 / float(m_shard)
            else:
                in_b = dram.tile([COUT, 2], f32)
                out_b = dram.tile([COUT, 2], f32)
                nc.gpsimd.dma_start(out=in_b[:], in_=sums[:])
                nc.gpsimd.collective_compute(
                    "AllReduce", mybir.AluOpType.add,
                    replica_groups=[list(range(n_cores))],
                    ins=[in_b.opt()], outs=[out_b.opt()])
                red = small.tile([COUT, 2], f32)
                nc.gpsimd.dma_start(out=red[:], in_=out_b[:])

                inv_m = 1.0 / float(n_cores * m_shard)
            mean = small.tile([COUT, 1], f32)
            nc.vector.tensor_scalar_mul(out=mean[:], in0=red[:, 0:1],
                                        scalar1=inv_m)
            ex2 = small.tile([COUT, 1], f32)
            nc.vector.tensor_scalar_mul(out=ex2[:], in0=red[:, 1:2],
                                        scalar1=inv_m)
            var = small.tile([COUT, 1], f32)
            nc.vector.tensor_tensor(out=var[:], in0=mean[:], in1=mean[:],
                                    op=mybir.AluOpType.mult)
            nc.vector.tensor_tensor(out=var[:], in0=ex2[:], in1=var[:],
                                    op=mybir.AluOpType.subtract)
            nc.vector.tensor_scalar_add(out=var[:], in0=var[:], scalar1=BN_EPS)
            std = small.tile([COUT, 1], f32)
            nc.scalar.activation(out=std[:], in_=var[:],
                                 func=mybir.ActivationFunctionType.Sqrt)
            rstd = small.tile([COUT, 1], f32)
            nc.vector.reciprocal(out=rstd[:], in_=std[:])

            st64 = small.tile([COUT, 2], f32)
            nc.vector.tensor_tensor(out=st64[:, 0:1], in0=gb_sb[:, 0:1],
                                    in1=rstd[:], op=mybir.AluOpType.mult)
            tmp = small.tile([COUT, 1], f32)
            nc.vector.tensor_tensor(out=tmp[:], in0=mean[:], in1=st64[:, 0:1],
                                    op=mybir.AluOpType.mult)
            nc.vector.tensor_tensor(out=st64[:, 1:2], in0=gb_sb[:, 1:2],
                                    in1=tmp[:], op=mybir.AluOpType.subtract)
            st128 = small.tile([P, 2], f32)
            nc.sync.dma_start(out=st128[0:COUT, :], in_=st64[:])
            nc.sync.dma_start(out=st128[COUT:2 * COUT, :], in_=st64[:])

            # ---------------- Phase 2 ----------------
            store = None
            for u in range(n_super):
                norm = sqn_pool.tile([P, SUPER // 2], MM_DT, tag="norm")
                if kraw:
                    nc.scalar.activation(
                        out=norm[:], in_=out_all[:, u * 512:(u + 1) * 512],
                        func=mybir.ActivationFunctionType.Copy)
                else:
                    nc.scalar.activation(
                        out=norm[:], in_=out_all[:, u * 512:(u + 1) * 512],
                        func=mybir.ActivationFunctionType.Relu,
                        scale=st128[:, 0:1], bias=st128[:, 1:2])
                if u % SCAT_SUPERS == 0:
                    store = store_pool.tile([P, SCAT_SUPERS * SUPER // 2],
                                            MM_DT, tag="store")
                soff = (u % SCAT_SUPERS) * (SUPER // 2)
                tp2 = psB.tile([P, SUPER // 2], MM_DT, tag="tp2")
                for i in range(4):
                    nc.tensor.transpose(
                        out=tp2[:, i * P:(i + 1) * P],
                        in_=norm[:, i * P:(i + 1) * P],
                        identity=ident[:])
                nc.vector.tensor_copy(out=store[:, soff:soff + 512], in_=tp2[:])
                if u % SCAT_SUPERS == SCAT_SUPERS - 1:
                    c0 = (u + 1 - SCAT_SUPERS) * 512
                    nc.sync.dma_start(
                        out=out_d.ap()[:, c0:c0 + SCAT_SUPERS * 512],
                        in_=store[:])

    nc.compile()
    return nc


def prepare_inputs(feats, weight, gamma, beta, in_idx, kidx, n_cores):
    in_idx = np.asarray(in_idx, np.int32)
    kidx = np.asarray(kidx, np.int32)
    feats = np.asarray(feats, np.float32)
    m = in_idx.shape[0]
    m_shard = m // n_cores
    n_in = feats.shape[0]
    gidx16, pos_all, plan, nt = build_schedule(
        in_idx, kidx, n_cores, m_shard, weight.shape[0], n_in)

    s_banks = plan["s_banks"]
    fb = np.zeros((s_banks * BROWS, feats.shape[1]), ml_dtypes.bfloat16)
    f16 = feats.astype(ml_dtypes.bfloat16)
    for b in range(s_banks):
        lo = b * BANK
        hi = min(lo + BANK, n_in)
        fb[b * BROWS:b * BROWS + (hi - lo)] = f16[lo:hi]

    w = np.asarray(weight, np.float32)
    kvol, cin, cout = w.shape
    wcat = np.zeros((cin, kvol, 2, P), np.float32)
    for k in range(kvol):
        wcat[:, k, 0, :cout] = w[k]
        wcat[:, k, 1, cout:2 * cout] = w[k]
    wcat = wcat.reshape(cin, kvol * 2 * P).astype(ml_dtypes.bfloat16)
    gb = np.stack([np.asarray(gamma, np.float32),
                   np.asarray(beta, np.float32)], axis=1)
    in_maps = [{
        "feats": fb, "wcat": wcat, "gb": gb,
        "gidx": np.ascontiguousarray(gidx16[c]),
    } for c in range(n_cores)]
    return in_maps, plan, nt, m_shard, n_in, pos_all


_CACHE = {}


def assemble_output(results, pos_all, m_shard, nt, n_cores):
    outs = []
    for c in range(n_cores):
        o = np.asarray(results[c]["out"])
        sched = o.reshape(P, nt, COUT).transpose(1, 0, 2) \
                 .reshape(nt * P, COUT).astype(np.float32)
        outs.append(sched[pos_all[c]])
    return np.concatenate(outs, 0)


def kernel(feats, weight, gamma, beta, in_idx, kidx):
    in_maps, plan, nt, m_shard, n_in, pos_all = prepare_inputs(
        feats, weight, gamma, beta, in_idx, kidx, N_CORES)

    key = (n_in, m_shard, nt,
           tuple(plan["gops"]),
           tuple(tuple(r) for rs in plan["runs"] for r in rs))
    nc = _CACHE.get(key)
    if nc is None:
        nc = build_program(n_in, m_shard, nt, plan, N_CORES)
        _CACHE[key] = nc

    res = bass_utils.run_bass_kernel_spmd(nc, in_maps,
                                          core_ids=list(range(N_CORES)))
    return assemble_output(res.results, pos_all, m_shard, nt, N_CORES)


# revision 8
# speedup vs baseline: 1.1259x; 1.1259x over previous
"""Sparse transposed-conv block (gather + per-offset GEMM + sync-BN + ReLU) on 8 TRN2 NeuronCores.

Strategy (data-parallel over output voxels, per the sharding hint):
 - Each core owns a contiguous block of M/8 output voxels; the full feats
   table is replicated (bf16, banked for int16 gather indices) and read with
   the bulk `dma_gather(transpose=True)` custom op, which lands gathered
   rows CHANNELS-ON-PARTITIONS — no PE transposes needed before the GEMM.
 - Host-side index prep only: per-shard voxels are sorted by
   (src-bank, kernel-offset); subgroup sizes are padded to 128 and to the
   max across cores so the single SPMD program fits every core's data.
   Pad slots gather a sentinel zero row, contributing exact zeros to the
   BN statistics.
 - Phase 1: transpose-gather (bf16) -> bf16 matmuls with [W_k|0]/[0|W_k]
   weight pairs accumulating a subtile-parity-packed [128, 512] PSUM
   supertile -> ACT copies it into an SBUF-resident fp16 pre-BN buffer
   while reduce-accumulating per-channel sums; a second ACT pass
   accumulates sums of squares.
 - Mid: [64,2] AllReduce across the 8 cores (sync-BN), scale/bias compute.
 - Phase 2: ACT fused relu(scale*x+bias) -> bf16 -> PE transpose back to
   voxel-major -> plain contiguous HWDGE writes into HBM laid out
   [128 part, voxel-block, 64ch]; the host un-permutes (each output voxel
   is written exactly once - the scatter is a collision-free permutation).
"""

import math
import os
import numpy as np
import ml_dtypes

import concourse.bass as bass
import concourse.bacc as bacc
import concourse.tile as tile
import concourse.mybir as mybir
from concourse import bass_utils
from concourse.masks import make_identity

P = 128
N_CORES = 8
BN_EPS = 1e-5

N_IN, M_FULL, CIN, COUT, KVOL = 200000, 600000, 128, 64, 4

BANK = 32767                 # real rows per bank (int16 sentinel at 32767)
BROWS = BANK + 1             # rows per bank incl sentinel
SUBS_PER_SUPER = 8           # 128-voxel subtiles per 1024-voxel supertile
SUPER = SUBS_PER_SUPER * P
MAX_OP = 896                 # max voxels per gather op: the SWDGE ring holds
                             # 1024 descriptors per queue and transpose-mode
                             # gathers emit ~1.125 descriptors per row
N_QUEUES = 4
SCAT_SUPERS = 2              # supertiles per contiguous output write (2KB/part)

MM_DT = mybir.dt.bfloat16
ACC_DT = mybir.dt.float16    # SBUF-resident pre-BN buffer dtype (values ~N(0,0.5))


def _wrap16(lst):
    """int16 index list -> [128, n/16] tile data (16-partition wrap,
    replicated for the 8 SWDGE cores)."""
    n = lst.shape[0]
    assert n % 16 == 0
    w = lst.reshape(n // 16, 16).T.astype(np.int16)   # [16, n/16]
    return np.tile(w, (8, 1))                          # [128, n/16]


def build_schedule(in_idx, kidx, n_cores, m_shard, kvol, n_in):
    """Returns (per-core gidx16 [C,128,NT*8], per-core sched-pos [C,m_shard],
    plan dict, NT)."""
    s_banks = math.ceil(n_in / BANK)

    counts = np.zeros((n_cores, s_banks, kvol), np.int64)
    orders = []
    for c in range(n_cores):
        r = np.arange(m_shard)
        k_sh = kidx[c * m_shard:(c + 1) * m_shard]
        src = in_idx[c * m_shard:(c + 1) * m_shard]
        sb = src // BANK
        order = np.lexsort((r, k_sh, sb))  # stable by (sb, k)
        orders.append(order)
        np.add.at(counts[c], (sb[order], k_sh[order]), 1)

    g_max = counts.max(axis=0)                       # [s_banks, kvol]
    g_pad = (np.ceil(g_max / P) * P).astype(np.int64)
    total = int(g_pad.sum())
    # extend the last nonzero subgroup so the schedule is store-tile-aligned
    batch = SCAT_SUPERS * SUPER
    total_al = math.ceil(total / batch) * batch
    nz = np.argwhere(g_pad > 0)
    ls, lk = nz[-1]
    g_pad[ls, lk] += total_al - total
    total = total_al
    nt = total // P

    # subgroup offsets in schedule order
    sg_off = np.zeros_like(g_pad)
    off = 0
    sg_list = []   # (sb, k, off, padded_len)
    for s in range(s_banks):
        for k in range(kvol):
            if g_pad[s, k] == 0:
                continue
            sg_off[s, k] = off
            sg_list.append((s, k, off, int(g_pad[s, k])))
            off += int(g_pad[s, k])

    # per-subtile k map -> per-supertile matmul runs
    sub_k = np.empty(nt, np.int64)
    for (s, k, o, ln) in sg_list:
        sub_k[o // P:(o + ln) // P] = k
    runs = []
    for u in range(total // SUPER):
        r = []
        ks = sub_k[u * SUBS_PER_SUPER:(u + 1) * SUBS_PER_SUPER]
        i = 0
        while i < SUBS_PER_SUPER:
            j = i
            while j < SUBS_PER_SUPER and ks[j] == ks[i]:
                j += 1
            r.append((int(ks[i]), i, j))
            i = j
        runs.append(r)

    # gather ops: src-bank-pure chunks (<= MAX_OP, 128-aligned); subgroups
    # sharing a src bank merge into one op up to the chunk limit
    gops = []   # (src_bank, sched_pos, n)
    for s in range(s_banks):
        ext = [(o, ln) for (s2, k, o, ln) in sg_list if s2 == s]
        if not ext:
            continue
        o0 = ext[0][0]
        ln = sum(e[1] for e in ext)
        assert o0 + ln == ext[-1][0] + ext[-1][1]
        p0 = o0
        while p0 < o0 + ln:
            n = min(MAX_OP, o0 + ln - p0)
            gops.append((s, p0, n))
            p0 += n

    # per-core int16 gather index lists + schedule positions
    gidx16 = np.empty((n_cores, P, nt * 8), np.int16)
    pos_all = np.empty((n_cores, m_shard), np.int64)
    for c in range(n_cores):
        order = orders[c]
        k_sh = kidx[c * m_shard:(c + 1) * m_shard]
        src = in_idx[c * m_shard:(c + 1) * m_shard]
        glist = np.full(total, BANK, np.int64)   # pad -> sentinel zero row
        k_o = k_sh[order]
        sb_o = (src // BANK)[order]
        key = sb_o * kvol + k_o
        uniq, inv, cnt = np.unique(key, return_inverse=True, return_counts=True)
        within = np.arange(m_shard) - np.concatenate([[0], np.cumsum(cnt)])[inv]
        pos = sg_off[sb_o, k_o] + within
        glist[pos] = (src % BANK)[order]
        pos_all[c][order] = pos
        gidx16[c] = _wrap16(glist)

    plan = dict(s_banks=s_banks, runs=runs, gops=gops, total=total)
    return gidx16, pos_all, plan, nt


def build_program(n_in, m_shard, nt, plan, n_cores):
    f32 = mybir.dt.float32
    i16 = mybir.dt.int16
    n_super = nt // SUBS_PER_SUPER
    s_banks = plan["s_banks"]
    runs, gops = plan["runs"], plan["gops"]

    nc = bacc.Bacc("TRN2", target_bir_lowering=False, debug=False,
                   num_devices=n_cores, num_swdge_queues=N_QUEUES)

    feats_d = nc.dram_tensor("feats", [s_banks * BROWS, CIN], MM_DT,
                             kind="ExternalInput")
    w_d = nc.dram_tensor("wcat", [CIN, KVOL * 2 * P], MM_DT, kind="ExternalInput")
    gb_d = nc.dram_tensor("gb", [COUT, 2], f32, kind="ExternalInput")
    gidx_d = nc.dram_tensor("gidx", [P, nt * 8], i16, kind="ExternalInput")
    out_d = nc.dram_tensor("out", [P, nt * COUT], MM_DT, kind="ExternalOutput")

    # static helper map: schedule subtile -> (gather op index, col offset in op)
    sub_op = {}
    for w, (s, p0, n) in enumerate(gops):
        for t in range(n // P):
            sub_op[p0 // P + t] = (w, t * P)

    with tile.TileContext(nc) as tc:
        with tc.tile_pool(name="const", bufs=1) as cpool, \
             tc.tile_pool(name="big", bufs=1) as big, \
             tc.tile_pool(name="gst", bufs=4) as gst_pool, \
             tc.tile_pool(name="gix", bufs=4) as gix_pool, \
             tc.tile_pool(name="sqn", bufs=2) as sqn_pool, \
             tc.tile_pool(name="store", bufs=2) as store_pool, \
             tc.tile_pool(name="small", bufs=1) as small, \
             tc.tile_pool(name="psA", bufs=3, space="PSUM") as psA, \
             tc.tile_pool(name="psB", bufs=2, space="PSUM") as psB, \
             tc.tile_pool(name="dram", bufs=2, space="DRAM") as dram:

            ident = cpool.tile([P, P], MM_DT)
            make_identity(nc, ident[:])
            w_sb = cpool.tile([CIN, KVOL * 2 * P], MM_DT)
            nc.sync.dma_start(out=w_sb[:], in_=w_d.ap())
            gb_sb = cpool.tile([COUT, 2], f32)
            nc.sync.dma_start(out=gb_sb[:], in_=gb_d.ap())

            out_all = big.tile([P, n_super * (SUPER // 2)], ACC_DT)
            macc = small.tile([P, n_super], f32)
            sacc = small.tile([P, n_super], f32)

            # ---------------- Phase 1 ----------------
            stage = {}   # op index -> staging tile [128 chan, n vox] bf16
            def issue_gather(w):
                s, p0, n = gops[w]
                gix = gix_pool.tile([P, MAX_OP // 16], i16, tag="gix")
                nc.sync.dma_start(out=gix[:, :n // 16],
                                  in_=gidx_d.ap()[:, p0 // 16:(p0 + n) // 16])
                gst = gst_pool.tile([P, MAX_OP], MM_DT, tag="gst")
                nc.gpsimd.dma_gather(
                    gst[:, :n].rearrange("p (o n) -> p o n", o=1),
                    feats_d.ap()[s * BROWS:(s + 1) * BROWS, :],
                    gix[:, :n // 16],
                    n, n, CIN,
                    transpose=True,
                    queue_num=w % N_QUEUES)
                stage[w] = gst

            next_op = 0
            for u in range(n_super):
                last_sub = (u + 1) * SUBS_PER_SUPER - 1
                while next_op < len(gops) and \
                        gops[next_op][1] // P <= last_sub:
                    issue_gather(next_op)
                    next_op += 1

                # out2[(c,j), blk*128 + p] = conv(voxel (2*blk+c)*128 + p)
                # start=True zeroes the whole 2KB PSUM bank (ZERO_REGION), so
                # only the first matmul of the supertile may set it; Tile
                # serializes same-bank ops in emission order.
                out2 = psA.tile([P, SUPER // 2], f32, tag="out2")
                mm_list = []   # (k, c, i0, nsub, op, col0)
                for (k, ss, se) in runs[u]:
                    for c in range(2):
                        subs = [t for t in range(ss, se) if t % 2 == c]
                        g = None
                        for t in subs:
                            w, col = sub_op[u * SUBS_PER_SUPER + t]
                            if g is not None and g[4] == w and \
                                    col == g[5] + g[3] * 2 * P:
                                g[3] += 1
                            else:
                                if g is not None:
                                    mm_list.append(tuple(g))
                                g = [k, c, t, 1, w, col]
                        if g is not None:
                            mm_list.append(tuple(g))
                kph = os.environ.get("KPH", "full")
                if kph in ("gather",):
                    continue
                for i, (k, c, t0, nsub, w, col0) in enumerate(mm_list):
                    gt = stage[w][:]
                    rhs = bass.AP(
                        gt.tensor, gt.offset + col0,
                        [gt.ap[0], [2 * P, nsub], [1, P]])
                    o0 = (t0 // 2) * P
                    nc.tensor.matmul(
                        out=out2[:, o0:o0 + nsub * P],
                        lhsT=w_sb[:, (k * 2 + c) * P:(k * 2 + c + 1) * P],
                        rhs=rhs,
                        start=(i == 0), stop=(i == len(mm_list) - 1),
                        skip_group_check=True)

                if kph == "mm":
                    continue
                nc.scalar.activation(
                    out=out_all[:, u * 512:(u + 1) * 512], in_=out2[:],
                    func=mybir.ActivationFunctionType.Copy,
                    accum_out=macc[:, u:u + 1])
                sq_sb = sqn_pool.tile([P, SUPER // 2], f32, tag="sqn")
                nc.scalar.activation(
                    out=sq_sb[:], in_=out2[:],
                    func=mybir.ActivationFunctionType.Square,
                    accum_out=sacc[:, u:u + 1])

            # ---------------- stats + AllReduce ----------------
            kph = os.environ.get("KPH", "full")
            kraw = os.environ.get("KRAW", "0") == "1"
            if kph in ("p1", "mm", "gather"):
                nc.compile()
                return nc
            stats = small.tile([P, 2], f32)
            nc.vector.reduce_sum(out=stats[:, 0:1], in_=macc[:],
                                 axis=mybir.AxisListType.X)
            nc.vector.reduce_sum(out=stats[:, 1:2], in_=sacc[:],
                                 axis=mybir.AxisListType.X)
            fold = small.tile([COUT, 2], f32)
            nc.sync.dma_start(out=fold[:], in_=stats[COUT:2 * COUT, :])
            sums = small.tile([COUT, 2], f32)
            nc.vector.tensor_add(out=sums[:], in0=stats[0:COUT, :], in1=fold[:])

            knocoll = os.environ.get("KNOCOLL", "0") == "1"
            if knocoll:
                red = sums
                inv_m = 1.0 / float(m_shard)
            else:
                in_b = dram.tile([COUT, 2], f32)
                out_b = dram.tile([COUT, 2], f32)
                nc.sync.dma_start(out=in_b[:], in_=sums[:])
                nc.gpsimd.collective_compute(
                    "AllReduce", mybir.AluOpType.add,
                    replica_groups=[list(range(n_cores))],
                    ins=[in_b.opt()], outs=[out_b.opt()])
                red = small.tile([COUT, 2], f32)
                nc.sync.dma_start(out=red[:], in_=out_b[:])

                inv_m = 1.0 / float(n_cores * m_shard)
            mean = small.tile([COUT, 1], f32)
            nc.vector.tensor_scalar_mul(out=mean[:], in0=red[:, 0:1],
                                        scalar1=inv_m)
            ex2 = small.tile([COUT, 1], f32)
            nc.vector.tensor_scalar_mul(out=ex2[:], in0=red[:, 1:2],
                                        scalar1=inv_m)
            var = small.tile([COUT, 1], f32)
            nc.vector.tensor_tensor(out=var[:], in0=mean[:], in1=mean[:],
                                    op=mybir.AluOpType.mult)
            nc.vector.tensor_tensor(out=var[:], in0=ex2[:], in1=var[:],
                                    op=mybir.AluOpType.subtract)
            nc.vector.tensor_scalar_add(out=var[:], in0=var[:], scalar1=BN_EPS)
            std = small.tile([COUT, 1], f32)
            nc.scalar.activation(out=std[:], in_=var[:],
                                 func=mybir.ActivationFunctionType.Sqrt)
            rstd = small.tile([COUT, 1], f32)
            nc.vector.reciprocal(out=rstd[:], in_=std[:])

            st64 = small.tile([COUT, 2], f32)
            nc.vector.tensor_tensor(out=st64[:, 0:1], in0=gb_sb[:, 0:1],
                                    in1=rstd[:], op=mybir.AluOpType.mult)
            tmp = small.tile([COUT, 1], f32)
            nc.vector.tensor_tensor(out=tmp[:], in0=mean[:], in1=st64[:, 0:1],
                                    op=mybir.AluOpType.mult)
            nc.vector.tensor_tensor(out=st64[:, 1:2], in0=gb_sb[:, 1:2],
                                    in1=tmp[:], op=mybir.AluOpType.subtract)
            st128 = small.tile([P, 2], f32)
            nc.sync.dma_start(out=st128[0:COUT, :], in_=st64[:])
            nc.sync.dma_start(out=st128[COUT:2 * COUT, :], in_=st64[:])

            # ---------------- Phase 2 ----------------
            if kph == "p1s":
                nc.compile()
                return nc
            store = None
            for u in range(n_super):
                norm = sqn_pool.tile([P, SUPER // 2], MM_DT, tag="norm")
                if kraw:
                    nc.scalar.activation(
                        out=norm[:], in_=out_all[:, u * 512:(u + 1) * 512],
                        func=mybir.ActivationFunctionType.Copy)
                else:
                    nc.scalar.activation(
                        out=norm[:], in_=out_all[:, u * 512:(u + 1) * 512],
                        func=mybir.ActivationFunctionType.Relu,
                        scale=st128[:, 0:1], bias=st128[:, 1:2])
                if u % SCAT_SUPERS == 0:
                    store = store_pool.tile([P, SCAT_SUPERS * SUPER // 2],
                                            MM_DT, tag="store")
                soff = (u % SCAT_SUPERS) * (SUPER // 2)
                tp2 = psB.tile([P, SUPER // 2], MM_DT, tag="tp2")
                for i in range(4):
                    nc.tensor.transpose(
                        out=tp2[:, i * P:(i + 1) * P],
                        in_=norm[:, i * P:(i + 1) * P],
                        identity=ident[:])
                nc.vector.tensor_copy(out=store[:, soff:soff + 512], in_=tp2[:])
                if u % SCAT_SUPERS == SCAT_SUPERS - 1:
                    c0 = (u + 1 - SCAT_SUPERS) * 512
                    nc.sync.dma_start(
                        out=out_d.ap()[:, c0:c0 + SCAT_SUPERS * 512],
                        in_=store[:])

    nc.compile()
    return nc


def prepare_inputs(feats, weight, gamma, beta, in_idx, kidx, n_cores):
    in_idx = np.asarray(in_idx, np.int32)
    kidx = np.asarray(kidx, np.int32)
    feats = np.asarray(feats, np.float32)
    m = in_idx.shape[0]
    m_shard = m // n_cores
    n_in = feats.shape[0]
    gidx16, pos_all, plan, nt = build_schedule(
        in_idx, kidx, n_cores, m_shard, weight.shape[0], n_in)

    s_banks = plan["s_banks"]
    fb = np.zeros((s_banks * BROWS, feats.shape[1]), ml_dtypes.bfloat16)
    f16 = feats.astype(ml_dtypes.bfloat16)
    for b in range(s_banks):
        lo = b * BANK
        hi = min(lo + BANK, n_in)
        fb[b * BROWS:b * BROWS + (hi - lo)] = f16[lo:hi]

    w = np.asarray(weight, np.float32)
    kvol, cin, cout = w.shape
    wcat = np.zeros((cin, kvol, 2, P), np.float32)
    for k in range(kvol):
        wcat[:, k, 0, :cout] = w[k]
        wcat[:, k, 1, cout:2 * cout] = w[k]
    wcat = wcat.reshape(cin, kvol * 2 * P).astype(ml_dtypes.bfloat16)
    gb = np.stack([np.asarray(gamma, np.float32),
                   np.asarray(beta, np.float32)], axis=1)
    in_maps = [{
        "feats": fb, "wcat": wcat, "gb": gb,
        "gidx": np.ascontiguousarray(gidx16[c]),
    } for c in range(n_cores)]
    return in_maps, plan, nt, m_shard, n_in, pos_all


_CACHE = {}


def assemble_output(results, pos_all, m_shard, nt, n_cores):
    outs = []
    for c in range(n_cores):
        o = np.asarray(results[c]["out"])
        sched = o.reshape(P, nt, COUT).transpose(1, 0, 2) \
                 .reshape(nt * P, COUT).astype(np.float32)
        outs.append(sched[pos_all[c]])
    return np.concatenate(outs, 0)


def kernel(feats, weight, gamma, beta, in_idx, kidx):
    in_maps, plan, nt, m_shard, n_in, pos_all = prepare_inputs(
        feats, weight, gamma, beta, in_idx, kidx, N_CORES)

    key = (n_in, m_shard, nt,
           tuple(plan["gops"]),
           tuple(tuple(r) for rs in plan["runs"] for r in rs))
    nc = _CACHE.get(key)
    if nc is None:
        nc = build_program(n_in, m_shard, nt, plan, N_CORES)
        _CACHE[key] = nc

    res = bass_utils.run_bass_kernel_spmd(nc, in_maps,
                                          core_ids=list(range(N_CORES)))
    return assemble_output(res.results, pos_all, m_shard, nt, N_CORES)
